# revision 25
# baseline (speedup 1.0000x reference)
"""Single-head self-attention (B=8, S=2048, D=K=V=1024) on 8 TRN2 NeuronCores.

Sharding: data-parallel over batch — one batch element per core. Each core
computes its full attention layer; no collectives.

Algebraic reduction: with q_i = x_i Wq + bq and k_j = x_j Wk + bk,
  q_i . k_j = x_i (Wq Wk^T) x_j^T + bq.(x_j Wk) + [terms constant over j]
and the j-constant terms cancel in softmax. So instead of projecting both q
and k (2 S·D·F matmul passes), precompute M = Wq Wk^T once (D·D·F) plus the
correction c_j = x_j . (Wk bq), and compute scores as x M x^T + c. bk is
mathematically unused.

Per-core dataflow (all matmuls bf16 with fp32 PSUM accumulation):
  phase 1: x --PE-transpose--> xT[d,s] (bf16, resident)
           WkT/WqT[f,d] via PE transposes; M[d1,d2] = WqT^T WkT (bf16)
           wkbq = rowsum(Wk * bq) on VectorE;  cT[j] = xT^T wkbq (PE)
           gT[d2,s] = M^T xT   (= (x M)^T, feature-major)
           v[s,f]  = x Wv + bv (natural layout)
  phase 2: per i-block of 512 queries:
           sT[j,i] = xT_j^T gT_i             (PE, contraction over d)
           eT = exp(sT*scale + c_j*scale)    (ScalarE, PSUM->SBUF, bf16)
           sums[i] = eT^T @ ones             (PE, per-partition result)
           o[i,:] = (eT_ic^T @ v) / sums     (PE + VectorE normalize)
"""

import numpy as np
from contextlib import ExitStack

import concourse.bass as bass
import concourse.tile as tile
from concourse import bacc, mybir
from concourse.bass_utils import run_bass_kernel_spmd
from concourse.masks import make_identity

P = 128
FP32 = mybir.dt.float32
BF16 = mybir.dt.bfloat16
AF = mybir.ActivationFunctionType

B, S_FULL, D_FULL, F_FULL = 8, 2048, 1024, 1024
N_CORES = 8


def build_attention(nc, S, D, F, repeat=1):
    scale = 1.0 / float(np.sqrt(F))
    ND, NF, NS = D // P, F // P, S // P
    SS = 512                 # s / i super-block width
    NSS = S // SS
    NI = S // SS
    NJ = NS                  # j blocks of 128
    VCW = min(F, 512)        # vd chunk width
    NV = F // VCW
    DCW = min(D, 512)        # d2 chunk width for M
    NDC = D // DCW
    NIC = SS // P            # i sub-chunks per i-block

    x = nc.dram_tensor("x", [S, D], FP32, kind="ExternalInput").ap()
    wq = nc.dram_tensor("wq", [D, F], FP32, kind="ExternalInput").ap()
    bq = nc.dram_tensor("bq", [F], FP32, kind="ExternalInput").ap()
    wk = nc.dram_tensor("wk", [D, F], FP32, kind="ExternalInput").ap()
    bk = nc.dram_tensor("bk", [F], FP32, kind="ExternalInput").ap()  # noqa: F841 — cancels in softmax
    wv = nc.dram_tensor("wv", [D, F], FP32, kind="ExternalInput").ap()
    bv = nc.dram_tensor("bv", [F], FP32, kind="ExternalInput").ap()
    out = nc.dram_tensor("out", [S, F], FP32, kind="ExternalOutput").ap()

    def bcast(vec, parts=P):
        return bass.AP(tensor=vec.tensor, offset=vec.offset,
                       ap=[[0, parts]] + list(vec.ap))

    with tile.TileContext(nc) as tc, ExitStack() as ctx:
        consts = ctx.enter_context(tc.tile_pool(name="consts", bufs=1))
        ident_bf = consts.tile([P, P], BF16)
        make_identity(nc, ident_bf)
        ones = consts.tile([P, 1], BF16)
        nc.vector.memset(ones, 1.0)
        bq_bc = consts.tile([P, F], FP32)
        bv_sb = consts.tile([P, F], FP32)

        kTx = ctx.enter_context(tc.tile_pool(name="xTp", bufs=1)).tile([P, ND, S], BF16)
        gT = ctx.enter_context(tc.tile_pool(name="gTp", bufs=1)).tile([P, ND, S], BF16)
        vv = ctx.enter_context(tc.tile_pool(name="vp", bufs=1)).tile([P, NS, F], BF16)
        csc = ctx.enter_context(tc.tile_pool(name="cp", bufs=1)).tile([P, NJ], FP32)

        # ---------------- Phase 1 ----------------
        def _phase1():
          with ExitStack() as ph1:
            wpool = ph1.enter_context(tc.tile_pool(name="wpool", bufs=1))
            wstage = ph1.enter_context(tc.tile_pool(name="wstage", bufs=2))
            wbstage = ph1.enter_context(tc.tile_pool(name="wbstage", bufs=2))
            smalls = ph1.enter_context(tc.tile_pool(name="smalls", bufs=1))
            xstage = ph1.enter_context(tc.tile_pool(name="xstage", bufs=3))
            xbstage = ph1.enter_context(tc.tile_pool(name="xbstage", bufs=3))
            ps_tr = ph1.enter_context(tc.tile_pool(name="ps_tr", bufs=3, space="PSUM"))
            ps_mm = ph1.enter_context(tc.tile_pool(name="ps_mm", bufs=4, space="PSUM"))
            ps_c = ph1.enter_context(tc.tile_pool(name="ps_c", bufs=1, space="PSUM"))

            wkT = wpool.tile([P, NF, D], BF16, tag="wkT")
            wqT = wpool.tile([P, NF, D], BF16, tag="wqT")
            wv_sb = wpool.tile([P, ND, F], BF16, tag="wv")
            m_sb = wpool.tile([P, ND, D], BF16, tag="m")
            wkbq = smalls.tile([P, ND], FP32, tag="wkbq")
            wkbq_bf = smalls.tile([P, ND], BF16, tag="wkbq_bf")

            def w_chunk(wap, dstT, do, with_wkbq):
                # dstT[f_in, f_out, d] <- transpose of W[d, f] chunk do
                st = wstage.tile([P, F], FP32, tag="wst")
                nc.sync.dma_start(st, wap[do * P:(do + 1) * P, :])
                wb = wbstage.tile([P, F], BF16, tag="wb")
                nc.gpsimd.tensor_copy(out=wb, in_=st)
                for h0 in range(0, NF, 4):
                    g = min(4, NF - h0)
                    pt = ps_tr.tile([P, 4, P], BF16, tag="pt")
                    for c in range(g):
                        nc.tensor.transpose(
                            pt[:, c, :], wb[:, (h0 + c) * P:(h0 + c + 1) * P],
                            ident_bf,
                        )
                    dst = dstT[:, h0:h0 + g, do * P:(do + 1) * P]
                    if (do + h0) % 2 == 0:
                        nc.scalar.copy(out=dst, in_=pt[:, :g, :])
                    else:
                        nc.vector.tensor_copy(out=dst, in_=pt[:, :g, :])
                if with_wkbq:
                    # wkbq[d] = sum_f W[d,f] * bq[f] on VectorE
                    tmp = wbstage.tile([P, F], FP32, tag="wkbq_tmp")
                    nc.vector.tensor_mul(tmp, st, bq_bc)
                    nc.vector.reduce_sum(
                        out=wkbq[:, do:do + 1], in_=tmp,
                        axis=mybir.AxisListType.X,
                    )

            def x_block(si):
                xs = xstage.tile([P, D], FP32, tag="xs")
                nc.sync.dma_start(xs, x[si * P:(si + 1) * P, :])
                xb = xbstage.tile([P, D], BF16, tag="xb")
                nc.gpsimd.tensor_copy(out=xb, in_=xs)
                for h0 in range(0, ND, 4):
                    g = min(4, ND - h0)
                    pt = ps_tr.tile([P, 4, P], BF16, tag="pt")
                    for c in range(g):
                        nc.tensor.transpose(
                            pt[:, c, :], xb[:, (h0 + c) * P:(h0 + c + 1) * P],
                            ident_bf,
                        )
                    dst = kTx[:, h0:h0 + g, si * P:(si + 1) * P]
                    if si % 2 == 0:
                        nc.scalar.copy(out=dst, in_=pt[:, :g, :])
                    else:
                        nc.vector.tensor_copy(out=dst, in_=pt[:, :g, :])

            # Interleave W-chunk and x-block processing so the PE has
            # transpose work throughout the DMA-bound startup window. Wk/Wq
            # first — they gate the critical path (M -> gT -> scores).
            xi = 0
            nc.scalar.dma_start(bq_bc, bcast(bq))
            for do in range(ND):
                w_chunk(wk, wkT, do, True)
                if xi < NS:
                    x_block(xi)
                    xi += 1
            nc.vector.tensor_copy(out=wkbq_bf, in_=wkbq)
            for do in range(ND):
                w_chunk(wq, wqT, do, False)
                if xi < NS:
                    x_block(xi)
                    xi += 1
            while xi < NS:
                x_block(xi)
                xi += 1

            # M[d1, d2] = sum_f Wq[d1,f] Wk[d2,f]
            for d1o in range(ND):
                for dc in range(NDC):
                    pmm = ps_mm.tile([P, DCW], FP32, tag="mm")
                    for fo in range(NF):
                        nc.tensor.matmul(
                            pmm,
                            wqT[:, fo, d1o * P:(d1o + 1) * P],
                            wkT[:, fo, dc * DCW:(dc + 1) * DCW],
                            start=(fo == 0),
                            stop=(fo == NF - 1),
                        )
                    nc.scalar.copy(out=m_sb[:, d1o, dc * DCW:(dc + 1) * DCW], in_=pmm)

            # cT[j] = sum_d xT[d, j] * wkbq[d]  (pre-scaled for the exp bias)
            pc = ps_c.tile([P, NJ], FP32, tag="c")
            for jb in range(NJ):
                for do in range(ND):
                    nc.tensor.matmul(
                        pc[:, jb:jb + 1],
                        kTx[:, do, jb * P:(jb + 1) * P],
                        wkbq_bf[:, do:do + 1],
                        start=(jb == 0 and do == 0),
                        stop=(jb == NJ - 1 and do == ND - 1),
                    )
            nc.vector.tensor_scalar_mul(csc, pc, scale)

            # gT[d2, s] = sum_d1 M[d1, d2] xT[d1, s]   (= (x M)^T)
            for ss in range(NSS):
                for d2o in range(ND):
                    pmm = ps_mm.tile([P, SS], FP32, tag="mm")
                    for d1o in range(ND):
                        nc.tensor.matmul(
                            pmm,
                            m_sb[:, d1o, d2o * P:(d2o + 1) * P],
                            kTx[:, d1o, ss * SS:(ss + 1) * SS],
                            start=(d1o == 0),
                            stop=(d1o == ND - 1),
                        )
                    nc.scalar.copy(out=gT[:, d2o, ss * SS:(ss + 1) * SS], in_=pmm)

            # Wv load + v[s, f] = x Wv + bv, emitted last: lowest priority, so
            # the v matmuls act as PE gap-filler behind the M/gT chain.
            nc.scalar.dma_start(bv_sb, bcast(bv))
            for do in range(ND):
                st = wstage.tile([P, F], FP32, tag="wst")
                nc.sync.dma_start(st, wv[do * P:(do + 1) * P, :])
                nc.gpsimd.tensor_copy(out=wv_sb[:, do, :], in_=st)
            for si in range(NS):
                for vc in range(NV):
                    c0 = vc * VCW
                    pmm = ps_mm.tile([P, VCW], FP32, tag="mm")
                    for do in range(ND):
                        nc.tensor.matmul(
                            pmm,
                            kTx[:, do, si * P:(si + 1) * P],
                            wv_sb[:, do, c0:c0 + VCW],
                            start=(do == 0),
                            stop=(do == ND - 1),
                        )
                    nc.vector.tensor_add(
                        out=vv[:, si, c0:c0 + VCW],
                        in0=pmm,
                        in1=bv_sb[:, c0:c0 + VCW],
                    )

        # ---------------- Phase 2: attention ----------------
        def _phase2():
          with ExitStack() as ph2:
            eTpool = ph2.enter_context(tc.tile_pool(name="eTpool", bufs=2))
            rpool = ph2.enter_context(tc.tile_pool(name="rpool", bufs=2))
            ostage = ph2.enter_context(tc.tile_pool(name="ostage", bufs=3))
            ps_s = ph2.enter_context(tc.tile_pool(name="ps_s", bufs=2, space="PSUM"))
            ps_st = ph2.enter_context(tc.tile_pool(name="ps_st", bufs=2, space="PSUM"))
            ps_av = ph2.enter_context(tc.tile_pool(name="ps_av", bufs=4, space="PSUM"))

            for ib in range(NI):
                eT = eTpool.tile([P, NJ, SS], BF16, tag="eT")
                psumT = ps_st.tile([P, NIC], FP32, tag="sumT")
                for jb in range(NJ):
                    ps = ps_s.tile([P, SS], FP32, tag="s")
                    for do in range(ND):
                        nc.tensor.matmul(
                            ps,
                            kTx[:, do, jb * P:(jb + 1) * P],
                            gT[:, do, ib * SS:(ib + 1) * SS],
                            start=(do == 0),
                            stop=(do == ND - 1),
                        )
                    nc.scalar.activation(
                        out=eT[:, jb, :], in_=ps, func=AF.Exp, scale=scale,
                        bias=csc[:, jb:jb + 1],
                    )
                    for ic in range(NIC):
                        # One PSUM accumulation group spans the whole [P, NIC]
                        # tile: start marks the full 2KB zero-region pending-
                        # zero, so each column's first write overwrites.
                        nc.tensor.matmul(
                            psumT[:, ic:ic + 1],
                            eT[:, jb, ic * P:(ic + 1) * P],
                            ones,
                            start=(jb == 0 and ic == 0),
                            stop=(jb == NJ - 1 and ic == NIC - 1),
                        )
                recip = rpool.tile([P, NIC], FP32, tag="recip")
                nc.vector.reciprocal(recip, psumT)
                for ic in range(NIC):
                    for vc in range(NV):
                        c0 = vc * VCW
                        po = ps_av.tile([P, VCW], FP32, tag="av")
                        for jb in range(NJ):
                            nc.tensor.matmul(
                                po,
                                eT[:, jb, ic * P:(ic + 1) * P],
                                vv[:, jb, c0:c0 + VCW],
                                start=(jb == 0),
                                stop=(jb == NJ - 1),
                            )
                        ot = ostage.tile([P, VCW], FP32, tag="ot")
                        nc.vector.tensor_scalar_mul(ot, po, recip[:, ic:ic + 1])
                        nc.sync.dma_start(
                            out[ib * SS + ic * P: ib * SS + (ic + 1) * P, c0:c0 + VCW],
                            ot,
                        )

        # `repeat` re-emits the whole computation; >1 used only for wall-clock
        # timing of the per-iteration device time.
        for _rep in range(repeat):
            _phase1()
            _phase2()
    return nc


_CACHE = {}


def _get_module():
    if "nc" not in _CACHE:
        nc = bacc.Bacc(
            "TRN2", target_bir_lowering=False, debug=False, num_devices=N_CORES
        )
        build_attention(nc, S_FULL, D_FULL, F_FULL)
        nc.compile()
        _CACHE["nc"] = nc
    return _CACHE["nc"]


def _in_maps(query, Wq, bq, Wk, bk, Wv, bv):
    def f32(a):
        return np.ascontiguousarray(np.asarray(a, dtype=np.float32))

    query, Wq, bq, Wk, bk, Wv, bv = map(f32, (query, Wq, bq, Wk, bk, Wv, bv))
    return [
        {
            "x": np.ascontiguousarray(query[b]),
            "wq": Wq,
            "bq": bq,
            "wk": Wk,
            "bk": bk,
            "wv": Wv,
            "bv": bv,
        }
        for b in range(B)
    ]


def kernel(query, Wq, bq, Wk, bk, Wv, bv):
    nc = _get_module()
    in_maps = _in_maps(query, Wq, bq, Wk, bk, Wv, bv)
    res = run_bass_kernel_spmd(nc, in_maps, core_ids=list(range(N_CORES)))
    return np.stack([r["out"] for r in res.results], axis=0)


# revision 31
# speedup vs baseline: 1.1843x; 1.1843x over previous
"""Single-head self-attention (B=8, S=2048, D=K=V=1024) on 8 TRN2 NeuronCores.

Sharding: data-parallel over batch — one batch element per core. Each core
computes its full attention layer; no collectives.

Algebraic reduction: with q_i = x_i Wq + bq and k_j = x_j Wk + bk,
  q_i . k_j = x_i (Wq Wk^T) x_j^T + bq.(x_j Wk) + [terms constant over j]
and the j-constant terms cancel in softmax. So instead of projecting both q
and k (2 S·D·F matmul passes), precompute M = Wq Wk^T once (D·D·F) plus the
correction c_j = x_j . (Wk bq), and compute scores as x M x^T + c. bk is
mathematically unused.

Per-core dataflow (all matmuls bf16 with fp32 PSUM accumulation):
  phase 1: x --PE-transpose--> xT[d,s] (bf16, resident)
           WkT/WqT[f,d] via PE transposes; M[d1,d2] = WqT^T WkT (bf16)
           wkbq = rowsum(Wk * bq) on VectorE;  cT[j] = xT^T wkbq (PE)
           gT[d2,s] = M^T xT   (= (x M)^T, feature-major)
           v[s,f]  = x Wv + bv (natural layout)
  phase 2: per i-block of 512 queries:
           sT[j,i] = xT_j^T gT_i             (PE, contraction over d)
           eT = exp(sT*scale + c_j*scale)    (ScalarE, PSUM->SBUF, bf16)
           sums[i] = eT^T @ ones             (PE, per-partition result)
           o[i,:] = (eT_ic^T @ v) / sums     (PE + VectorE normalize)
"""

import numpy as np
from contextlib import ExitStack

import concourse.bass as bass
import concourse.tile as tile
from concourse import bacc, mybir
from concourse.bass_utils import run_bass_kernel_spmd
from concourse.masks import make_identity

P = 128
FP32 = mybir.dt.float32
BF16 = mybir.dt.bfloat16
AF = mybir.ActivationFunctionType

B, S_FULL, D_FULL, F_FULL = 8, 2048, 1024, 1024
N_CORES = 8


def build_attention(nc, S, D, F, repeat=1):
    scale = 1.0 / float(np.sqrt(F))
    ND, NF, NS = D // P, F // P, S // P
    SS = 512                 # s / i super-block width
    NSS = S // SS
    NI = S // SS
    NJ = NS                  # j blocks of 128
    VCW = min(F, 512)        # vd chunk width
    NV = F // VCW
    DCW = min(D, 512)        # d2 chunk width for M
    NDC = D // DCW
    NIC = SS // P            # i sub-chunks per i-block

    x = nc.dram_tensor("x", [S, D], FP32, kind="ExternalInput").ap()
    wq = nc.dram_tensor("wq", [D, F], FP32, kind="ExternalInput").ap()
    bq = nc.dram_tensor("bq", [F], FP32, kind="ExternalInput").ap()
    wk = nc.dram_tensor("wk", [D, F], FP32, kind="ExternalInput").ap()
    bk = nc.dram_tensor("bk", [F], FP32, kind="ExternalInput").ap()  # noqa: F841 — cancels in softmax
    wv = nc.dram_tensor("wv", [D, F], FP32, kind="ExternalInput").ap()
    bv = nc.dram_tensor("bv", [F], FP32, kind="ExternalInput").ap()
    out = nc.dram_tensor("out", [S, F], FP32, kind="ExternalOutput").ap()

    def bcast(vec, parts=P):
        return bass.AP(tensor=vec.tensor, offset=vec.offset,
                       ap=[[0, parts]] + list(vec.ap))

    with tile.TileContext(nc) as tc, ExitStack() as ctx:
        consts = ctx.enter_context(tc.tile_pool(name="consts", bufs=1))
        ident_bf = consts.tile([P, P], BF16)
        make_identity(nc, ident_bf)
        ones = consts.tile([P, 1], BF16)
        nc.vector.memset(ones, 1.0)
        bq_sb = consts.tile([P, NF], FP32)
        bq_bf = consts.tile([P, NF], BF16)
        bv_sb = consts.tile([P, F], FP32)

        kTx = ctx.enter_context(tc.tile_pool(name="xTp", bufs=1)).tile([P, ND, S], BF16)
        gT = ctx.enter_context(tc.tile_pool(name="gTp", bufs=1)).tile([P, ND, S], BF16)
        vv = ctx.enter_context(tc.tile_pool(name="vp", bufs=1)).tile([P, NS, F], BF16)
        csc = ctx.enter_context(tc.tile_pool(name="cp", bufs=1)).tile([P, NJ], FP32)

        # ---------------- Phase 1 ----------------
        def _phase1():
          with ExitStack() as ph1:
            wpool = ph1.enter_context(tc.tile_pool(name="wpool", bufs=1))
            wstage = ph1.enter_context(tc.tile_pool(name="wstage", bufs=2))
            wbstage = ph1.enter_context(tc.tile_pool(name="wbstage", bufs=2))
            smalls = ph1.enter_context(tc.tile_pool(name="smalls", bufs=1))
            xstage = ph1.enter_context(tc.tile_pool(name="xstage", bufs=3))
            xbstage = ph1.enter_context(tc.tile_pool(name="xbstage", bufs=3))
            ps_tr = ph1.enter_context(tc.tile_pool(name="ps_tr", bufs=3, space="PSUM"))
            ps_mm = ph1.enter_context(tc.tile_pool(name="ps_mm", bufs=4, space="PSUM"))
            ps_c = ph1.enter_context(tc.tile_pool(name="ps_c", bufs=1, space="PSUM"))

            wkT = wpool.tile([P, NF, D], BF16, tag="wkT")
            wqT = wpool.tile([P, NF, D], BF16, tag="wqT")
            wv_sb = wpool.tile([P, ND, F], BF16, tag="wv")
            m_sb = wpool.tile([P, ND, D], BF16, tag="m")
            wkbq_bf = smalls.tile([P, ND], BF16, tag="wkbq_bf")

            def w_chunk(wap, dstT, do):
                # dstT[f_in, f_out, d] <- transpose of W[d, f] chunk do
                st = wstage.tile([P, F], FP32, tag="wst")
                nc.sync.dma_start(st, wap[do * P:(do + 1) * P, :])
                wb = wbstage.tile([P, F], BF16, tag="wb")
                nc.gpsimd.tensor_copy(out=wb, in_=st)
                for h0 in range(0, NF, 4):
                    g = min(4, NF - h0)
                    pt = ps_tr.tile([P, 4, P], BF16, tag="pt")
                    for c in range(g):
                        nc.tensor.transpose(
                            pt[:, c, :], wb[:, (h0 + c) * P:(h0 + c + 1) * P],
                            ident_bf,
                        )
                    dst = dstT[:, h0:h0 + g, do * P:(do + 1) * P]
                    if (do + h0) % 2 == 0:
                        nc.scalar.copy(out=dst, in_=pt[:, :g, :])
                    else:
                        nc.vector.tensor_copy(out=dst, in_=pt[:, :g, :])

            def x_block(si):
                xs = xstage.tile([P, D], FP32, tag="xs")
                nc.sync.dma_start(xs, x[si * P:(si + 1) * P, :])
                xb = xbstage.tile([P, D], BF16, tag="xb")
                nc.scalar.copy(out=xb, in_=xs)
                for h0 in range(0, ND, 4):
                    g = min(4, ND - h0)
                    pt = ps_tr.tile([P, 4, P], BF16, tag="pt")
                    for c in range(g):
                        nc.tensor.transpose(
                            pt[:, c, :], xb[:, (h0 + c) * P:(h0 + c + 1) * P],
                            ident_bf,
                        )
                    dst = kTx[:, h0:h0 + g, si * P:(si + 1) * P]
                    if si % 2 == 0:
                        nc.scalar.copy(out=dst, in_=pt[:, :g, :])
                    else:
                        nc.vector.tensor_copy(out=dst, in_=pt[:, :g, :])

            # Interleave W-chunk and x-block processing so the PE has
            # transpose work throughout the DMA-bound startup window. Wk/Wq
            # first — they gate the critical path (M -> gT -> scores).
            xi = 0
            nc.scalar.dma_start(bq_sb, bq.rearrange("(fo fi) -> fi fo", fi=P))
            nc.vector.tensor_copy(out=bq_bf, in_=bq_sb)
            for do in range(ND):
                w_chunk(wk, wkT, do)
                if xi < NS:
                    x_block(xi)
                    xi += 1
            # wkbq[d] = sum_f Wk[d,f] bq[f] via tiny PE matmuls off wkT:
            # out[d-chunk, 1] accumulates over f-chunks; fills startup gaps.
            pwkbq = ps_c.tile([P, ND], FP32, tag="c")
            for do in range(ND):
                for fo in range(NF):
                    nc.tensor.matmul(
                        pwkbq[:, do:do + 1],
                        wkT[:, fo, do * P:(do + 1) * P],
                        bq_bf[:, fo:fo + 1],
                        start=(do == 0 and fo == 0),
                        stop=(do == ND - 1 and fo == NF - 1),
                    )
            nc.vector.tensor_copy(out=wkbq_bf, in_=pwkbq)
            for do in range(ND):
                w_chunk(wq, wqT, do)
                if xi < NS:
                    x_block(xi)
                    xi += 1
            while xi < NS:
                x_block(xi)
                xi += 1

            # M[d1, d2] = sum_f Wq[d1,f] Wk[d2,f]
            for d1o in range(ND):
                for dc in range(NDC):
                    pmm = ps_mm.tile([P, DCW], FP32, tag="mm")
                    for fo in range(NF):
                        nc.tensor.matmul(
                            pmm,
                            wqT[:, fo, d1o * P:(d1o + 1) * P],
                            wkT[:, fo, dc * DCW:(dc + 1) * DCW],
                            start=(fo == 0),
                            stop=(fo == NF - 1),
                        )
                    nc.scalar.copy(out=m_sb[:, d1o, dc * DCW:(dc + 1) * DCW], in_=pmm)

            # cT[j] = sum_d xT[d, j] * wkbq[d]  (pre-scaled for the exp bias)
            pc = ps_c.tile([P, NJ], FP32, tag="c")
            for jb in range(NJ):
                for do in range(ND):
                    nc.tensor.matmul(
                        pc[:, jb:jb + 1],
                        kTx[:, do, jb * P:(jb + 1) * P],
                        wkbq_bf[:, do:do + 1],
                        start=(jb == 0 and do == 0),
                        stop=(jb == NJ - 1 and do == ND - 1),
                    )
            nc.vector.tensor_scalar_mul(csc, pc, scale)

            # gT[d2, s] = sum_d1 M[d1, d2] xT[d1, s]   (= (x M)^T)
            for ss in range(NSS):
                for d2o in range(ND):
                    pmm = ps_mm.tile([P, SS], FP32, tag="mm")
                    for d1o in range(ND):
                        nc.tensor.matmul(
                            pmm,
                            m_sb[:, d1o, d2o * P:(d2o + 1) * P],
                            kTx[:, d1o, ss * SS:(ss + 1) * SS],
                            start=(d1o == 0),
                            stop=(d1o == ND - 1),
                        )
                    nc.scalar.copy(out=gT[:, d2o, ss * SS:(ss + 1) * SS], in_=pmm)

            # Wv load + v[s, f] = x Wv + bv, emitted last: lowest priority, so
            # the v matmuls act as PE gap-filler behind the M/gT chain.
            nc.scalar.dma_start(bv_sb, bcast(bv))
            for do in range(ND):
                st = wstage.tile([P, F], FP32, tag="wst")
                nc.sync.dma_start(st, wv[do * P:(do + 1) * P, :])
                nc.gpsimd.tensor_copy(out=wv_sb[:, do, :], in_=st)
            for si in range(NS):
                for vc in range(NV):
                    c0 = vc * VCW
                    pmm = ps_mm.tile([P, VCW], FP32, tag="mm")
                    for do in range(ND):
                        nc.tensor.matmul(
                            pmm,
                            kTx[:, do, si * P:(si + 1) * P],
                            wv_sb[:, do, c0:c0 + VCW],
                            start=(do == 0),
                            stop=(do == ND - 1),
                        )
                    nc.vector.tensor_add(
                        out=vv[:, si, c0:c0 + VCW],
                        in0=pmm,
                        in1=bv_sb[:, c0:c0 + VCW],
                    )

        # ---------------- Phase 2: attention ----------------
        def _phase2():
          with ExitStack() as ph2:
            eTpool = ph2.enter_context(tc.tile_pool(name="eTpool", bufs=2))
            rpool = ph2.enter_context(tc.tile_pool(name="rpool", bufs=2))
            ostage = ph2.enter_context(tc.tile_pool(name="ostage", bufs=3))
            ps_s = ph2.enter_context(tc.tile_pool(name="ps_s", bufs=2, space="PSUM"))
            ps_st = ph2.enter_context(tc.tile_pool(name="ps_st", bufs=2, space="PSUM"))
            ps_av = ph2.enter_context(tc.tile_pool(name="ps_av", bufs=4, space="PSUM"))

            for ib in range(NI):
                eT = eTpool.tile([P, NJ, SS], BF16, tag="eT")
                psumT = ps_st.tile([P, NIC], FP32, tag="sumT")
                for jb in range(NJ):
                    ps = ps_s.tile([P, SS], FP32, tag="s")
                    for do in range(ND):
                        nc.tensor.matmul(
                            ps,
                            kTx[:, do, jb * P:(jb + 1) * P],
                            gT[:, do, ib * SS:(ib + 1) * SS],
                            start=(do == 0),
                            stop=(do == ND - 1),
                        )
                    nc.scalar.activation(
                        out=eT[:, jb, :], in_=ps, func=AF.Exp, scale=scale,
                        bias=csc[:, jb:jb + 1],
                    )
                    for ic in range(NIC):
                        # One PSUM accumulation group spans the whole [P, NIC]
                        # tile: start marks the full 2KB zero-region pending-
                        # zero, so each column's first write overwrites.
                        nc.tensor.matmul(
                            psumT[:, ic:ic + 1],
                            eT[:, jb, ic * P:(ic + 1) * P],
                            ones,
                            start=(jb == 0 and ic == 0),
                            stop=(jb == NJ - 1 and ic == NIC - 1),
                        )
                recip = rpool.tile([P, NIC], FP32, tag="recip")
                nc.vector.reciprocal(recip, psumT)
                for ic in range(NIC):
                    for vc in range(NV):
                        c0 = vc * VCW
                        po = ps_av.tile([P, VCW], FP32, tag="av")
                        for jb in range(NJ):
                            nc.tensor.matmul(
                                po,
                                eT[:, jb, ic * P:(ic + 1) * P],
                                vv[:, jb, c0:c0 + VCW],
                                start=(jb == 0),
                                stop=(jb == NJ - 1),
                            )
                        ot = ostage.tile([P, VCW], FP32, tag="ot")
                        nc.vector.tensor_scalar_mul(ot, po, recip[:, ic:ic + 1])
                        nc.sync.dma_start(
                            out[ib * SS + ic * P: ib * SS + (ic + 1) * P, c0:c0 + VCW],
                            ot,
                        )

        # `repeat` re-emits the whole computation; >1 used only for wall-clock
        # timing of the per-iteration device time.
        for _rep in range(repeat):
            _phase1()
            _phase2()
    return nc


_CACHE = {}


def _get_module():
    if "nc" not in _CACHE:
        nc = bacc.Bacc(
            "TRN2", target_bir_lowering=False, debug=False, num_devices=N_CORES
        )
        build_attention(nc, S_FULL, D_FULL, F_FULL)
        nc.compile()
        _CACHE["nc"] = nc
    return _CACHE["nc"]


def _in_maps(query, Wq, bq, Wk, bk, Wv, bv):
    def f32(a):
        return np.ascontiguousarray(np.asarray(a, dtype=np.float32))

    query, Wq, bq, Wk, bk, Wv, bv = map(f32, (query, Wq, bq, Wk, bk, Wv, bv))
    return [
        {
            "x": np.ascontiguousarray(query[b]),
            "wq": Wq,
            "bq": bq,
            "wk": Wk,
            "bk": bk,
            "wv": Wv,
            "bv": bv,
        }
        for b in range(B)
    ]


def kernel(query, Wq, bq, Wk, bk, Wv, bv):
    nc = _get_module()
    in_maps = _in_maps(query, Wq, bq, Wk, bk, Wv, bv)
    res = run_bass_kernel_spmd(nc, in_maps, core_ids=list(range(N_CORES)))
    return np.stack([r["out"] for r in res.results], axis=0)
